# revision 8
# baseline (speedup 1.0000x reference)
"""ChebConv (K=4) Trainium2 kernel: 8-core row-sharded SpMM + dense contraction.

v1 dataflow (all-bf16, SBUF-resident):
  Table layout: two regions, H0 = per-core rows [0,4096) (8*4096=32768 rows)
  and H1 = per-core rows [4096,6272) (8*2176=17408 rows). The int16 gather
  index splits exactly at the H0/H1 tensor boundary, and each AllGather step
  is two collectives (H0 issued after tile 31 so it overlaps the tail of the
  producing step; H1 after tile 48).
  Per 128-row tile: dma_gather edge columns (bf16, 512B rows) from H0/H1
  tables with per-core actual counts in a Q7 register (trailing pads are
  idx=-1 and never fetched); scatter-reduce into the tile's rows with
  per-128-slot indicator matmuls (mv = (iota==rl)*v, bf16) accumulating in
  PSUM. The Chebyshev subtract is folded into the same PSUM group via a
  leading (-identity) @ x_{k-2} matmul; x_{k-2} is read from the SBUF-
  resident bf16 x_k buffers (4 x 3.2MB), which also feed the contraction.
  Contraction (interleaved with step 3 per 4-tile block): PE-transpose
  x_k tiles to feature-major bf16, out[b].T = sum_k W_k.T @ x_k[b].T with
  bf16 weights, bias via tensor_scalar, fp32 out.
"""

import sys

sys.path.insert(0, "/opt/trn_rl_repo")

import numpy as np
import ml_dtypes

V = 50000
E = 800000
B, CIN, COUT, K = 4, 64, 128, 4
NC = 8
VC = V // NC              # 6250
VCP = 6272                # 49*128 padded rows per core
TILES = VCP // 128        # 49
F = B * CIN               # 256
H0R = 4096                # per-core rows in table region H0 (32 tiles)
H1R = VCP - H0R           # 2176 per-core rows in H1 (17 tiles)
NH0 = NC * H0R            # 32768
NH1 = NC * H1R            # 17408
T0 = H0R // 128           # 32 tiles in H0
BF16 = ml_dtypes.bfloat16
SEGCAP = 1536


# ---------------- host-side preprocessing ----------------

def _ceil128(n):
    return max(128, -(-n // 128) * 128)


def _segs_of(total, base):
    """Chop [base, base+total) into SEGCAP pieces -> [(off, len), ...]."""
    out = []
    done = 0
    while done < total:
        m = min(SEGCAP, total - done)
        out.append((base + done, m))
        done += m
    return out


def preprocess(rows, cols, vals):
    """Per (core, tile): lo/hi column-sorted halves (lo -> H0 table,
    hi -> H1 table), static sizes padded to per-tile max over cores,
    per-core actual counts per gather segment, gather indices (pads=-1),
    per-chunk indicator metadata (bf16)."""
    rows = np.asarray(rows).astype(np.int64)
    cols = np.asarray(cols).astype(np.int64)
    vals = np.asarray(vals, dtype=np.float32)

    cc = cols // VC
    rr = cols % VC
    is_lo = rr < H0R
    remap = np.where(is_lo, cc * H0R + rr, cc * H1R + (rr - H0R))

    per_core = []
    nlo = np.zeros((NC, TILES), np.int64)
    nhi = np.zeros((NC, TILES), np.int64)
    for c in range(NC):
        m = (rows >= c * VC) & (rows < (c + 1) * VC)
        r = rows[m] - c * VC
        cg = remap[m]
        lo = is_lo[m]
        vv = vals[m]
        t_of = r // 128
        tiles = []
        for t in range(TILES):
            sel = t_of == t
            rt, ct, vt, lt = r[sel] - t * 128, cg[sel], vv[sel], lo[sel]

            def srt(rr2, cc2, vv2):
                o = np.argsort(cc2, kind="stable")
                return rr2[o], cc2[o], vv2[o]

            tl = srt(rt[lt], ct[lt], vt[lt])
            th = srt(rt[~lt], ct[~lt], vt[~lt])
            nlo[c, t] = len(tl[0])
            nhi[c, t] = len(th[0])
            tiles.append((tl, th))
        per_core.append(tiles)

    NLOS = [_ceil128(int(nlo[:, t].max())) for t in range(TILES)]
    NHIS = [_ceil128(int(nhi[:, t].max())) for t in range(TILES)]
    CPTS = [(NLOS[t] + NHIS[t]) // 128 for t in range(TILES)]
    SPT16S = [(NLOS[t] + NHIS[t]) // 16 for t in range(TILES)]
    NCH = sum(CPTS)
    NIDX = sum(SPT16S)
    # static segment table: per tile, lo segs then hi segs
    SEGS = []          # list per tile: [(off, len, is_hi), ...]
    for t in range(TILES):
        s = [(o, n, False) for o, n in _segs_of(NLOS[t], 0)]
        s += [(o, n, True) for o, n in _segs_of(NHIS[t], NLOS[t])]
        SEGS.append(s)
    NSEG = sum(len(s) for s in SEGS)

    cores = []
    for c in range(NC):
        idx = np.full((128, NIDX), -1, dtype=np.int16)
        rl = np.zeros((128, NCH), dtype=np.float32)
        v1 = np.zeros((128, NCH), dtype=np.float32)
        cnt = np.zeros((1, NSEG), dtype=np.int32)
        io = 0
        ch0 = 0
        si = 0
        for t in range(TILES):
            (rlo, clo, vlo), (rhi, chi, vhi) = per_core[c][t]
            NLO, NHI = NLOS[t], NHIS[t]
            SPT16, CPT = SPT16S[t], CPTS[t]
            ns = NLO + NHI
            rw = np.zeros(ns, np.float32)
            cw = np.full(ns, -1, np.int64)
            vw = np.zeros(ns, np.float32)
            n = len(rlo)
            rw[:n], cw[:n], vw[:n] = rlo, clo, vlo
            nh = len(rhi)
            rw[NLO : NLO + nh], cw[NLO : NLO + nh], vw[NLO : NLO + nh] = rhi, chi, vhi
            # per-seg actual counts (>=1: dummy valid idx 0 if empty)
            for off, ln, is_hi in SEGS[t]:
                na = n if not is_hi else nh
                base = 0 if not is_hi else NLO
                k = int(np.clip(na - (off - base), 0, ln))
                if k == 0:
                    cw[off] = 0
                    k = 1
                cnt[0, si] = k
                si += 1
            # gather idx, 16-wrapped, replicated across the 8 Q7 groups
            w = cw.reshape(SPT16, 16).T.astype(np.int16)
            idx[:, io : io + SPT16] = np.tile(w, (8, 1))
            rl[:, ch0 : ch0 + CPT] = rw.reshape(CPT, 128).T
            v1[:, ch0 : ch0 + CPT] = vw.reshape(CPT, 128).T
            io += SPT16
            ch0 += CPT
        cores.append(dict(idx=idx, cnt=cnt, rl=rl, v1=v1, v2=2.0 * v1))
    return cores, NLOS, NHIS, SEGS


def host_inputs(x, lap_rows, lap_cols, lap_vals, weight, bias):
    x0 = np.ascontiguousarray(np.transpose(x, (2, 0, 1)).reshape(V, F)).astype(np.float32)
    x0g = x0.astype(BF16)
    tab0h0 = np.zeros((NH0, F), dtype=BF16)
    tab0h1 = np.zeros((NH1, F), dtype=BF16)
    slices = []
    for c in range(NC):
        sl = np.zeros((VCP, F), dtype=BF16)
        sl[:VC] = x0g[c * VC : (c + 1) * VC]
        tab0h0[c * H0R : (c + 1) * H0R] = sl[:H0R]
        tab0h1[c * H1R : (c + 1) * H1R] = sl[H0R:]
        slices.append(sl)
    cores, NLOS, NHIS, SEGS = preprocess(lap_rows, lap_cols, lap_vals)

    consts = np.zeros((128, 384), dtype=np.float32)
    consts[:, 0:128] = np.eye(128)
    consts[:, 128:256] = -np.eye(128)
    consts[:, 256:384] = np.broadcast_to(np.arange(128, dtype=np.float32)[None, :], (128, 128))
    consts = consts.astype(BF16)

    wlo = np.zeros((128, K * COUT), np.float32)
    whi = np.zeros((128, K * COUT), np.float32)
    for k in range(K):
        wlo[0:64, k * COUT : (k + 1) * COUT] = weight[k]
        whi[64:128, k * COUT : (k + 1) * COUT] = weight[k]
    bias_t = np.asarray(bias, np.float32).reshape(128, 1)
    in_maps = []
    for c in range(NC):
        in_maps.append(
            dict(
                x0slice=slices[c],
                tab0h0=tab0h0,
                tab0h1=tab0h1,
                idx=cores[c]["idx"],
                cnt=cores[c]["cnt"],
                rl=cores[c]["rl"],
                v1=cores[c]["v1"],
                v2=cores[c]["v2"],
                consts=consts,
                wlo=wlo.astype(BF16),
                whi=whi.astype(BF16),
                bias=bias_t,
            )
        )
    return in_maps, NLOS, NHIS, SEGS


# ---------------- device module ----------------

_CACHE = {}
_SCOPES = False


def build_module(NLOS, NHIS, SEGS, sim=False, NQ=4):
    NLOS, NHIS = tuple(NLOS), tuple(NHIS)
    key = (NLOS, NHIS, sim, NQ, _SCOPES)
    if key in _CACHE:
        return _CACHE[key]
    from concourse import bass, mybir, bacc
    import concourse.tile as tile

    CPTS = [(NLOS[t] + NHIS[t]) // 128 for t in range(TILES)]
    SPT16S = [(NLOS[t] + NHIS[t]) // 16 for t in range(TILES)]
    CHOFF = np.concatenate([[0], np.cumsum(CPTS)]).astype(int)
    IDXOFF = np.concatenate([[0], np.cumsum(SPT16S)]).astype(int)
    NCH = int(CHOFF[-1])
    NIDX = int(IDXOFF[-1])
    NSEG = sum(len(s) for s in SEGS)
    SEGOFF = np.concatenate([[0], np.cumsum([len(s) for s in SEGS])]).astype(int)
    CPT_MAX = max(CPTS)
    f32, i16, i32 = mybir.dt.float32, mybir.dt.int16, mybir.dt.int32
    gdt = mybir.dt.bfloat16

    nc = bacc.Bacc("TRN2", target_bir_lowering=False, debug=False,
                   num_devices=1 if sim else NC, num_swdge_queues=NQ)

    x0slice = nc.dram_tensor("x0slice", [VCP, F], gdt, kind="ExternalInput")
    tab0h0 = nc.dram_tensor("tab0h0", [NH0, F], gdt, kind="ExternalInput")
    tab0h1 = nc.dram_tensor("tab0h1", [NH1, F], gdt, kind="ExternalInput")
    idx_in = nc.dram_tensor("idx", [128, NIDX], i16, kind="ExternalInput")
    cnt_in = nc.dram_tensor("cnt", [1, NSEG], i32, kind="ExternalInput")
    rl_in = nc.dram_tensor("rl", [128, NCH], f32, kind="ExternalInput")
    v1_in = nc.dram_tensor("v1", [128, NCH], f32, kind="ExternalInput")
    v2_in = nc.dram_tensor("v2", [128, NCH], f32, kind="ExternalInput")
    consts_in = nc.dram_tensor("consts", [128, 384], gdt, kind="ExternalInput")
    wlo_in = nc.dram_tensor("wlo", [128, K * COUT], gdt, kind="ExternalInput")
    whi_in = nc.dram_tensor("whi", [128, K * COUT], gdt, kind="ExternalInput")
    bias_in = nc.dram_tensor("bias", [128, 1], f32, kind="ExternalInput")
    out_t = nc.dram_tensor("out", [B, COUT, VCP], f32, kind="ExternalOutput")

    creg = [nc.alloc_register(mybir.EngineType.Pool, name=f"cnt{i}") for i in range(2)]

    with tile.TileContext(nc) as tc:
        with (
            tc.tile_pool(name="pers", bufs=1) as pers,
            tc.tile_pool(name="gpool", bufs=3) as gpool,
            tc.tile_pool(name="mval", bufs=20) as mvpool,
            tc.tile_pool(name="spmm_ps", bufs=3, space="PSUM") as pspool,
            tc.tile_pool(name="tp_ps", bufs=2, space="PSUM") as tppool,
            tc.tile_pool(name="out_ps", bufs=2, space="PSUM") as popool,
            tc.tile_pool(name="xt", bufs=10) as xtpool,
            tc.tile_pool(name="obuf", bufs=3) as obpool,
            tc.tile_pool(name="dram", bufs=1, space="DRAM") as dram,
        ):
            import contextlib
            scope = nc.named_scope if _SCOPES else (lambda name: contextlib.nullcontext())

            idx_t = pers.tile([128, NIDX], i16)
            nc.sync.dma_start(idx_t[:], idx_in[:])
            cnt_t = pers.tile([1, NSEG], i32)
            nc.sync.dma_start(cnt_t[:], cnt_in[:])
            rl_t = pers.tile([128, NCH], f32)
            nc.sync.dma_start(rl_t[:], rl_in[:])
            v1_t = pers.tile([128, NCH], f32)
            nc.sync.dma_start(v1_t[:], v1_in[:])
            v2_t = pers.tile([128, NCH], f32)
            nc.sync.dma_start(v2_t[:], v2_in[:])
            consts_t = pers.tile([128, 384], gdt)
            nc.sync.dma_start(consts_t[:], consts_in[:])
            wlo_t = pers.tile([128, K * COUT], gdt)
            nc.sync.dma_start(wlo_t[:], wlo_in[:])
            whi_t = pers.tile([128, K * COUT], gdt)
            nc.sync.dma_start(whi_t[:], whi_in[:])
            bias_t = pers.tile([128, 1], f32)
            nc.sync.dma_start(bias_t[:], bias_in[:])
            ident = consts_t[:, 0:128]
            negident = consts_t[:, 128:256]
            iota = consts_t[:, 256:384]

            # SBUF-resident x_k (bf16): [p, t, f]
            xk_sb = [pers.tile([128, TILES * F], gdt, name=f"xk{k}") for k in range(K)]
            nc.sync.dma_start(
                xk_sb[0][:].rearrange("p (t f) -> p t f", f=F),
                x0slice[:].rearrange("(t p) f -> p t f", p=128),
            )
            # zero-fill gather pool bufs once (pad slots are never written by
            # the count-limited gathers; stale-garbage * mv=0 must stay finite)
            for _ in range(3):
                zt = gpool.tile([128, CPT_MAX * F], gdt, tag="G")
                nc.vector.memset(zt[:], 0.0)

            bncg = [dram.tile([VCP, F], gdt, name=f"bncg{i}", tag=f"bncg{i}") for i in range(2)]
            tabs = []
            for i in range(2):
                shared = "Local" if sim else "Shared"
                tabs.append((
                    dram.tile([NH0, F], gdt, name=f"tab{i+1}h0", tag=f"tab{i+1}h0", addr_space=shared),
                    dram.tile([NH1, F], gdt, name=f"tab{i+1}h1", tag=f"tab{i+1}h1", addr_space=shared),
                ))

            # ---------- contraction ----------
            def phase2_vblock(vb):
                with scope("p2"):
                    nt = min(4, TILES - vb * 4)
                    v0, nv = vb * 4 * 128, nt * 128
                    xts = []
                    for k in range(K):
                        xt_lo = xtpool.tile([128, 512], gdt, tag="xtlo")
                        xt_hi = xtpool.tile([128, 512], gdt, tag="xthi")
                        for q in range(nt):
                            for h in range(2):
                                dst = xt_lo if h == 0 else xt_hi
                                nc.sync.dma_start(
                                    dst[:, q * 128 : (q + 1) * 128],
                                    xk_sb[k][:, (4 * vb + q) * F + h * 128 : (4 * vb + q) * F + (h + 1) * 128],
                                    transpose=True,
                                )
                        xts.append((xt_lo, xt_hi))
                    for b in range(B):
                        h, off = divmod(b, 2)
                        off *= 64
                        wt = wlo_t if off == 0 else whi_t
                        po = popool.tile([128, 512], f32, space="PSUM")
                        for k in range(K):
                            xt = xts[k][h]
                            nc.tensor.matmul(
                                out=po[:, :nv], lhsT=wt[off : off + 64, k * COUT : (k + 1) * COUT],
                                rhs=xt[off : off + 64, :nv], start=(k == 0), stop=(k == K - 1),
                            )
                        ob = obpool.tile([128, 512], f32, tag="ob")
                        nc.any.tensor_scalar_add(ob[:, :nv], po[:, :nv], bias_t[:, 0:1])
                        nc.sync.dma_start(out_t[b, :, v0 : v0 + nv], ob[:, :nv])

            # ---------- SpMM steps ----------
            for k in (1, 2, 3):
                src_lo, src_hi = (tab0h0, tab0h1) if k == 1 else tabs[k - 2]
                vmeta = v1_t if k == 1 else v2_t
                prev = None if k == 1 else xk_sb[k - 2]
                ctx = scope(f"step{k}")
                ctx.__enter__()
                for t in range(TILES):
                    NLO, NHI, CPT = NLOS[t], NHIS[t], CPTS[t]
                    gt = gpool.tile([128, CPT_MAX * F], gdt, tag="G")
                    c0 = int(IDXOFF[t])
                    for si, (off, n, hi) in enumerate(SEGS[t]):
                        sg = int(SEGOFF[t]) + si
                        reg = creg[sg % 2]
                        nc.gpsimd.reg_load(reg, cnt_t[0:1, sg : sg + 1])
                        nc.gpsimd.dma_gather(
                            out_ap=gt[:, off * 2 : (off + n) * 2].rearrange(
                                "p (j f) -> p j f", f=F),
                            in_ap=(src_hi if hi else src_lo)[:],
                            idxs_ap=idx_t[:, c0 + off // 16 : c0 + (off + n) // 16],
                            num_idxs=n, num_idxs_reg=reg, elem_size=F,
                            single_packet=False, queue_num=(t * 3 + si) % NQ,
                        )
                    ps = pspool.tile([128, F], f32, space="PSUM")
                    if k > 1:
                        nc.tensor.matmul(
                            out=ps[:], lhsT=negident, rhs=prev[:, t * F : (t + 1) * F],
                            start=True, stop=False,
                        )
                    for j in range(CPT):
                        ch = int(CHOFF[t]) + j
                        mv = mvpool.tile([128, 128], gdt)
                        nc.vector.tensor_scalar(
                            out=mv[:], in0=iota,
                            scalar1=rl_t[:, ch : ch + 1], scalar2=vmeta[:, ch : ch + 1],
                            op0=mybir.AluOpType.is_equal, op1=mybir.AluOpType.mult,
                        )
                        nc.tensor.matmul(
                            out=ps[:], lhsT=mv[:], rhs=gt[:, j * F : (j + 1) * F],
                            start=(j == 0 and k == 1), stop=(j == CPT - 1),
                        )
                    nc.scalar.copy(out=xk_sb[k][:, t * F : (t + 1) * F], in_=ps[:])
                    if k < 3:
                        nc.sync.dma_start(
                            bncg[k - 1][t * 128 : (t + 1) * 128, :],
                            xk_sb[k][:, t * F : (t + 1) * F])
                        if t == T0 - 1 or t == TILES - 1:
                            h = 0 if t == T0 - 1 else 1
                            agin = bncg[k - 1][0:H0R, :] if h == 0 else bncg[k - 1][H0R:VCP, :]
                            agout = tabs[k - 1][h]
                            with scope(f"ag{k}{'ab'[h]}"):
                                if sim:
                                    nc.sync.dma_start(
                                        agout[0 : (H0R if h == 0 else H1R), :], agin)
                                else:
                                    nc.gpsimd.collective_compute(
                                        "AllGather", mybir.AluOpType.bypass,
                                        replica_groups=[list(range(NC))],
                                        ins=[agin.opt()], outs=[agout[:].opt()],
                                    )
                    if k == 3 and t % 4 == 3:
                        phase2_vblock(t // 4)
                ctx.__exit__(None, None, None)

            phase2_vblock(12)  # tail tile 48

    nc.compile()
    _CACHE[key] = nc
    return nc


# ---------------- entry point ----------------

def kernel(x, lap_rows, lap_cols, lap_vals, weight, bias):
    from concourse.bass_utils import run_bass_kernel_spmd

    x = np.asarray(x, np.float32)
    weight = np.asarray(weight, np.float32)
    bias = np.asarray(bias, np.float32)
    in_maps, NLOS, NHIS, SEGS = host_inputs(x, lap_rows, lap_cols, lap_vals, weight, bias)
    nc = build_module(NLOS, NHIS, SEGS)
    res = run_bass_kernel_spmd(nc, in_maps, core_ids=list(range(NC)))
    out = np.empty((B, COUT, V), np.float32)
    for c in range(NC):
        out[:, :, c * VC : (c + 1) * VC] = res.results[c]["out"][:, :, :VC]
    return out


# revision 9
# speedup vs baseline: 2.3823x; 2.3823x over previous
"""ChebConv (K=4) Trainium2 kernel: 8-core row-sharded SpMM + dense contraction.

v1 dataflow (all-bf16, SBUF-resident):
  Table layout: two regions, H0 = per-core rows [0,4096) (8*4096=32768 rows)
  and H1 = per-core rows [4096,6272) (8*2176=17408 rows). The int16 gather
  index splits exactly at the H0/H1 tensor boundary, and each AllGather step
  is two collectives (H0 issued after tile 31 so it overlaps the tail of the
  producing step; H1 after tile 48).
  Per 128-row tile: dma_gather edge columns (bf16, 512B rows) from H0/H1
  tables with per-core actual counts in a Q7 register (trailing pads are
  idx=-1 and never fetched); scatter-reduce into the tile's rows with
  per-128-slot indicator matmuls (mv = (iota==rl)*v, bf16) accumulating in
  PSUM. The Chebyshev subtract is folded into the same PSUM group via a
  leading (-identity) @ x_{k-2} matmul; x_{k-2} is read from the SBUF-
  resident bf16 x_k buffers (4 x 3.2MB), which also feed the contraction.
  Contraction (interleaved with step 3 per 4-tile block): PE-transpose
  x_k tiles to feature-major bf16, out[b].T = sum_k W_k.T @ x_k[b].T with
  bf16 weights, bias via tensor_scalar, fp32 out.
"""

import sys

sys.path.insert(0, "/opt/trn_rl_repo")

import numpy as np
import ml_dtypes

V = 50000
E = 800000
B, CIN, COUT, K = 4, 64, 128, 4
NC = 8
VC = V // NC              # 6250
VCP = 6272                # 49*128 padded rows per core
TILES = VCP // 128        # 49
F = B * CIN               # 256
H0R = 4096                # per-core rows in table region H0 (32 tiles)
H1R = VCP - H0R           # 2176 per-core rows in H1 (17 tiles)
NH0 = NC * H0R            # 32768
NH1 = NC * H1R            # 17408
T0 = H0R // 128           # 32 tiles in H0
BF16 = ml_dtypes.bfloat16
SEGCAP = 1536


# ---------------- host-side preprocessing ----------------

def _ceil128(n):
    return max(128, -(-n // 128) * 128)


def _segs_of(total, base):
    """Chop [base, base+total) into SEGCAP pieces -> [(off, len), ...]."""
    out = []
    done = 0
    while done < total:
        m = min(SEGCAP, total - done)
        out.append((base + done, m))
        done += m
    return out


def preprocess(rows, cols, vals):
    """Per (core, tile): lo/hi column-sorted halves (lo -> H0 table,
    hi -> H1 table), static sizes padded to per-tile max over cores,
    per-core actual counts per gather segment, gather indices (pads=-1),
    per-chunk indicator metadata (bf16)."""
    rows = np.asarray(rows).astype(np.int64)
    cols = np.asarray(cols).astype(np.int64)
    vals = np.asarray(vals, dtype=np.float32)

    cc = cols // VC
    rr = cols % VC
    is_lo = rr < H0R
    remap = np.where(is_lo, cc * H0R + rr, cc * H1R + (rr - H0R))

    per_core = []
    nlo = np.zeros((NC, TILES), np.int64)
    nhi = np.zeros((NC, TILES), np.int64)
    for c in range(NC):
        m = (rows >= c * VC) & (rows < (c + 1) * VC)
        r = rows[m] - c * VC
        cg = remap[m]
        lo = is_lo[m]
        vv = vals[m]
        t_of = r // 128
        tiles = []
        for t in range(TILES):
            sel = t_of == t
            rt, ct, vt, lt = r[sel] - t * 128, cg[sel], vv[sel], lo[sel]

            def srt(rr2, cc2, vv2):
                o = np.argsort(cc2, kind="stable")
                return rr2[o], cc2[o], vv2[o]

            tl = srt(rt[lt], ct[lt], vt[lt])
            th = srt(rt[~lt], ct[~lt], vt[~lt])
            nlo[c, t] = len(tl[0])
            nhi[c, t] = len(th[0])
            tiles.append((tl, th))
        per_core.append(tiles)

    NLOS = [_ceil128(int(nlo[:, t].max())) for t in range(TILES)]
    NHIS = [_ceil128(int(nhi[:, t].max())) for t in range(TILES)]
    CPTS = [(NLOS[t] + NHIS[t]) // 128 for t in range(TILES)]
    SPT16S = [(NLOS[t] + NHIS[t]) // 16 for t in range(TILES)]
    NCH = sum(CPTS)
    NIDX = sum(SPT16S)
    # static segment table: per tile, lo segs then hi segs
    SEGS = []          # list per tile: [(off, len, is_hi), ...]
    for t in range(TILES):
        s = [(o, n, False) for o, n in _segs_of(NLOS[t], 0)]
        s += [(o, n, True) for o, n in _segs_of(NHIS[t], NLOS[t])]
        SEGS.append(s)
    NSEG = sum(len(s) for s in SEGS)

    cores = []
    for c in range(NC):
        idx = np.full((128, NIDX), -1, dtype=np.int16)
        rl = np.zeros((128, NCH), dtype=np.float32)
        v1 = np.zeros((128, NCH), dtype=np.float32)
        cnt = np.zeros((1, NSEG), dtype=np.int32)
        io = 0
        ch0 = 0
        si = 0
        for t in range(TILES):
            (rlo, clo, vlo), (rhi, chi, vhi) = per_core[c][t]
            NLO, NHI = NLOS[t], NHIS[t]
            SPT16, CPT = SPT16S[t], CPTS[t]
            ns = NLO + NHI
            rw = np.zeros(ns, np.float32)
            cw = np.full(ns, -1, np.int64)
            vw = np.zeros(ns, np.float32)
            n = len(rlo)
            rw[:n], cw[:n], vw[:n] = rlo, clo, vlo
            nh = len(rhi)
            rw[NLO : NLO + nh], cw[NLO : NLO + nh], vw[NLO : NLO + nh] = rhi, chi, vhi
            # per-seg actual counts (>=1: dummy valid idx 0 if empty)
            for off, ln, is_hi in SEGS[t]:
                na = n if not is_hi else nh
                base = 0 if not is_hi else NLO
                k = int(np.clip(na - (off - base), 0, ln))
                if k == 0:
                    cw[off] = 0
                    k = 1
                cnt[0, si] = k
                si += 1
            # gather idx, 16-wrapped, replicated across the 8 Q7 groups
            w = cw.reshape(SPT16, 16).T.astype(np.int16)
            idx[:, io : io + SPT16] = np.tile(w, (8, 1))
            rl[:, ch0 : ch0 + CPT] = rw.reshape(CPT, 128).T
            v1[:, ch0 : ch0 + CPT] = vw.reshape(CPT, 128).T
            io += SPT16
            ch0 += CPT
        cores.append(dict(idx=idx, cnt=cnt, rl=rl, v1=v1, v2=2.0 * v1))
    return cores, NLOS, NHIS, SEGS


def host_inputs(x, lap_rows, lap_cols, lap_vals, weight, bias):
    x0 = np.ascontiguousarray(np.transpose(x, (2, 0, 1)).reshape(V, F)).astype(np.float32)
    x0g = x0.astype(BF16)
    tab0h0 = np.zeros((NH0, F), dtype=BF16)
    tab0h1 = np.zeros((NH1, F), dtype=BF16)
    slices = []
    for c in range(NC):
        sl = np.zeros((VCP, F), dtype=BF16)
        sl[:VC] = x0g[c * VC : (c + 1) * VC]
        tab0h0[c * H0R : (c + 1) * H0R] = sl[:H0R]
        tab0h1[c * H1R : (c + 1) * H1R] = sl[H0R:]
        slices.append(sl)
    cores, NLOS, NHIS, SEGS = preprocess(lap_rows, lap_cols, lap_vals)

    consts = np.zeros((128, 384), dtype=np.float32)
    consts[:, 0:128] = np.eye(128)
    consts[:, 128:256] = -np.eye(128)
    consts[:, 256:384] = np.broadcast_to(np.arange(128, dtype=np.float32)[None, :], (128, 128))
    consts = consts.astype(BF16)

    wlo = np.zeros((128, K * COUT), np.float32)
    whi = np.zeros((128, K * COUT), np.float32)
    for k in range(K):
        wlo[0:64, k * COUT : (k + 1) * COUT] = weight[k]
        whi[64:128, k * COUT : (k + 1) * COUT] = weight[k]
    bias_t = np.asarray(bias, np.float32).reshape(128, 1)
    in_maps = []
    for c in range(NC):
        in_maps.append(
            dict(
                x0slice=slices[c],
                tab0h0=tab0h0,
                tab0h1=tab0h1,
                idx=cores[c]["idx"],
                cnt=cores[c]["cnt"],
                rl=cores[c]["rl"],
                v1=cores[c]["v1"],
                v2=cores[c]["v2"],
                consts=consts,
                wlo=wlo.astype(BF16),
                whi=whi.astype(BF16),
                bias=bias_t,
            )
        )
    return in_maps, NLOS, NHIS, SEGS


# ---------------- device module ----------------

_CACHE = {}
_SCOPES = False


def build_module(NLOS, NHIS, SEGS, sim=False, NQ=4):
    NLOS, NHIS = tuple(NLOS), tuple(NHIS)
    key = (NLOS, NHIS, sim, NQ, _SCOPES)
    if key in _CACHE:
        return _CACHE[key]
    from concourse import bass, mybir, bacc
    import concourse.tile as tile

    CPTS = [(NLOS[t] + NHIS[t]) // 128 for t in range(TILES)]
    SPT16S = [(NLOS[t] + NHIS[t]) // 16 for t in range(TILES)]
    CHOFF = np.concatenate([[0], np.cumsum(CPTS)]).astype(int)
    IDXOFF = np.concatenate([[0], np.cumsum(SPT16S)]).astype(int)
    NCH = int(CHOFF[-1])
    NIDX = int(IDXOFF[-1])
    NSEG = sum(len(s) for s in SEGS)
    SEGOFF = np.concatenate([[0], np.cumsum([len(s) for s in SEGS])]).astype(int)
    CPT_MAX = max(CPTS)
    f32, i16, i32 = mybir.dt.float32, mybir.dt.int16, mybir.dt.int32
    gdt = mybir.dt.bfloat16

    nc = bacc.Bacc("TRN2", target_bir_lowering=False, debug=False,
                   num_devices=1 if sim else NC, num_swdge_queues=NQ)

    x0slice = nc.dram_tensor("x0slice", [VCP, F], gdt, kind="ExternalInput")
    tab0h0 = nc.dram_tensor("tab0h0", [NH0, F], gdt, kind="ExternalInput")
    tab0h1 = nc.dram_tensor("tab0h1", [NH1, F], gdt, kind="ExternalInput")
    idx_in = nc.dram_tensor("idx", [128, NIDX], i16, kind="ExternalInput")
    cnt_in = nc.dram_tensor("cnt", [1, NSEG], i32, kind="ExternalInput")
    rl_in = nc.dram_tensor("rl", [128, NCH], f32, kind="ExternalInput")
    v1_in = nc.dram_tensor("v1", [128, NCH], f32, kind="ExternalInput")
    v2_in = nc.dram_tensor("v2", [128, NCH], f32, kind="ExternalInput")
    consts_in = nc.dram_tensor("consts", [128, 384], gdt, kind="ExternalInput")
    wlo_in = nc.dram_tensor("wlo", [128, K * COUT], gdt, kind="ExternalInput")
    whi_in = nc.dram_tensor("whi", [128, K * COUT], gdt, kind="ExternalInput")
    bias_in = nc.dram_tensor("bias", [128, 1], f32, kind="ExternalInput")
    out_t = nc.dram_tensor("out", [B, COUT, VCP], f32, kind="ExternalOutput")

    creg = [nc.alloc_register(mybir.EngineType.Pool, name=f"cnt{i}") for i in range(2)]

    with tile.TileContext(nc) as tc:
        with (
            tc.tile_pool(name="pers", bufs=1) as pers,
            tc.tile_pool(name="gpool", bufs=3) as gpool,
            tc.tile_pool(name="mval", bufs=20) as mvpool,
            tc.tile_pool(name="spmm_ps", bufs=3, space="PSUM") as pspool,
            tc.tile_pool(name="tp_ps", bufs=2, space="PSUM") as tppool,
            tc.tile_pool(name="out_ps", bufs=2, space="PSUM") as popool,
            tc.tile_pool(name="xt", bufs=10) as xtpool,
            tc.tile_pool(name="obuf", bufs=3) as obpool,
            tc.tile_pool(name="dram", bufs=1, space="DRAM") as dram,
        ):
            import contextlib
            scope = nc.named_scope if _SCOPES else (lambda name: contextlib.nullcontext())

            idx_t = pers.tile([128, NIDX], i16)
            nc.sync.dma_start(idx_t[:], idx_in[:])
            cnt_t = pers.tile([1, NSEG], i32)
            nc.sync.dma_start(cnt_t[:], cnt_in[:])
            rl_t = pers.tile([128, NCH], f32)
            nc.sync.dma_start(rl_t[:], rl_in[:])
            v1_t = pers.tile([128, NCH], f32)
            nc.sync.dma_start(v1_t[:], v1_in[:])
            v2_t = pers.tile([128, NCH], f32)
            nc.sync.dma_start(v2_t[:], v2_in[:])
            consts_t = pers.tile([128, 384], gdt)
            nc.sync.dma_start(consts_t[:], consts_in[:])
            wlo_t = pers.tile([128, K * COUT], gdt)
            nc.sync.dma_start(wlo_t[:], wlo_in[:])
            whi_t = pers.tile([128, K * COUT], gdt)
            nc.sync.dma_start(whi_t[:], whi_in[:])
            bias_t = pers.tile([128, 1], f32)
            nc.sync.dma_start(bias_t[:], bias_in[:])
            ident = consts_t[:, 0:128]
            negident = consts_t[:, 128:256]
            iota = consts_t[:, 256:384]

            # SBUF-resident x_k (bf16): [p, t, f]
            xk_sb = [pers.tile([128, TILES * F], gdt, name=f"xk{k}") for k in range(K)]
            nc.sync.dma_start(
                xk_sb[0][:].rearrange("p (t f) -> p t f", f=F),
                x0slice[:].rearrange("(t p) f -> p t f", p=128),
            )
            # zero-fill gather pool bufs once (pad slots are never written by
            # the count-limited gathers; stale-garbage * mv=0 must stay finite)
            for _ in range(3):
                zt = gpool.tile([128, CPT_MAX * F], gdt, tag="G")
                nc.vector.memset(zt[:], 0.0)

            bncg = [dram.tile([VCP, F], gdt, name=f"bncg{i}", tag=f"bncg{i}") for i in range(2)]
            tabs = []
            for i in range(2):
                shared = "Local" if sim else "Shared"
                tabs.append((
                    dram.tile([NH0, F], gdt, name=f"tab{i+1}h0", tag=f"tab{i+1}h0", addr_space=shared),
                    dram.tile([NH1, F], gdt, name=f"tab{i+1}h1", tag=f"tab{i+1}h1", addr_space=shared),
                ))

            # ---------- contraction ----------
            def phase2_vblock(vb):
                with scope("p2"):
                    nt = min(4, TILES - vb * 4)
                    v0, nv = vb * 4 * 128, nt * 128
                    xts = []
                    for k in range(K):
                        xt_lo = xtpool.tile([128, 512], gdt, tag="xtlo")
                        xt_hi = xtpool.tile([128, 512], gdt, tag="xthi")
                        for q in range(nt):
                            for h in range(2):
                                tp = tppool.tile([128, 128], gdt, space="PSUM")
                                nc.tensor.transpose(
                                    out=tp[:],
                                    in_=xk_sb[k][:, (4 * vb + q) * F + h * 128 : (4 * vb + q) * F + (h + 1) * 128],
                                    identity=ident,
                                )
                                dst = xt_lo if h == 0 else xt_hi
                                nc.any.tensor_copy(out=dst[:, q * 128 : (q + 1) * 128], in_=tp[:])
                        xts.append((xt_lo, xt_hi))
                    for b in range(B):
                        h, off = divmod(b, 2)
                        off *= 64
                        wt = wlo_t if off == 0 else whi_t
                        po = popool.tile([128, 512], f32, space="PSUM")
                        for k in range(K):
                            xt = xts[k][h]
                            nc.tensor.matmul(
                                out=po[:, :nv], lhsT=wt[off : off + 64, k * COUT : (k + 1) * COUT],
                                rhs=xt[off : off + 64, :nv], start=(k == 0), stop=(k == K - 1),
                            )
                        ob = obpool.tile([128, 512], f32, tag="ob")
                        nc.any.tensor_scalar_add(ob[:, :nv], po[:, :nv], bias_t[:, 0:1])
                        nc.sync.dma_start(out_t[b, :, v0 : v0 + nv], ob[:, :nv])

            # ---------- SpMM steps ----------
            for k in (1, 2, 3):
                src_lo, src_hi = (tab0h0, tab0h1) if k == 1 else tabs[k - 2]
                vmeta = v1_t if k == 1 else v2_t
                prev = None if k == 1 else xk_sb[k - 2]
                ctx = scope(f"step{k}")
                ctx.__enter__()
                for t in range(TILES):
                    NLO, NHI, CPT = NLOS[t], NHIS[t], CPTS[t]
                    gt = gpool.tile([128, CPT_MAX * F], gdt, tag="G")
                    c0 = int(IDXOFF[t])
                    for si, (off, n, hi) in enumerate(SEGS[t]):
                        sg = int(SEGOFF[t]) + si
                        reg = creg[sg % 2]
                        nc.gpsimd.reg_load(reg, cnt_t[0:1, sg : sg + 1])
                        nc.gpsimd.dma_gather(
                            out_ap=gt[:, off * 2 : (off + n) * 2].rearrange(
                                "p (j f) -> p j f", f=F),
                            in_ap=(src_hi if hi else src_lo)[:],
                            idxs_ap=idx_t[:, c0 + off // 16 : c0 + (off + n) // 16],
                            num_idxs=n, num_idxs_reg=reg, elem_size=F,
                            single_packet=False, queue_num=(t * 3 + si) % NQ,
                        )
                    ps = pspool.tile([128, F], f32, space="PSUM")
                    if k > 1:
                        nc.tensor.matmul(
                            out=ps[:], lhsT=negident, rhs=prev[:, t * F : (t + 1) * F],
                            start=True, stop=False,
                        )
                    for j in range(CPT):
                        ch = int(CHOFF[t]) + j
                        mv = mvpool.tile([128, 128], gdt)
                        nc.vector.tensor_scalar(
                            out=mv[:], in0=iota,
                            scalar1=rl_t[:, ch : ch + 1], scalar2=vmeta[:, ch : ch + 1],
                            op0=mybir.AluOpType.is_equal, op1=mybir.AluOpType.mult,
                        )
                        nc.tensor.matmul(
                            out=ps[:], lhsT=mv[:], rhs=gt[:, j * F : (j + 1) * F],
                            start=(j == 0 and k == 1), stop=(j == CPT - 1),
                        )
                    nc.scalar.copy(out=xk_sb[k][:, t * F : (t + 1) * F], in_=ps[:])
                    if k < 3:
                        nc.sync.dma_start(
                            bncg[k - 1][t * 128 : (t + 1) * 128, :],
                            xk_sb[k][:, t * F : (t + 1) * F])
                        if t == T0 - 1 or t == TILES - 1:
                            h = 0 if t == T0 - 1 else 1
                            agin = bncg[k - 1][0:H0R, :] if h == 0 else bncg[k - 1][H0R:VCP, :]
                            agout = tabs[k - 1][h]
                            with scope(f"ag{k}{'ab'[h]}"):
                                if sim:
                                    nc.sync.dma_start(
                                        agout[0 : (H0R if h == 0 else H1R), :], agin)
                                else:
                                    nc.gpsimd.collective_compute(
                                        "AllGather", mybir.AluOpType.bypass,
                                        replica_groups=[list(range(NC))],
                                        ins=[agin.opt()], outs=[agout[:].opt()],
                                    )
                    if k == 3 and t % 4 == 3:
                        phase2_vblock(t // 4)
                ctx.__exit__(None, None, None)

            phase2_vblock(12)  # tail tile 48

    nc.compile()
    _CACHE[key] = nc
    return nc


# ---------------- entry point ----------------

def kernel(x, lap_rows, lap_cols, lap_vals, weight, bias):
    from concourse.bass_utils import run_bass_kernel_spmd

    x = np.asarray(x, np.float32)
    weight = np.asarray(weight, np.float32)
    bias = np.asarray(bias, np.float32)
    in_maps, NLOS, NHIS, SEGS = host_inputs(x, lap_rows, lap_cols, lap_vals, weight, bias)
    nc = build_module(NLOS, NHIS, SEGS)
    res = run_bass_kernel_spmd(nc, in_maps, core_ids=list(range(NC)))
    out = np.empty((B, COUT, V), np.float32)
    for c in range(NC):
        out[:, :, c * VC : (c + 1) * VC] = res.results[c]["out"][:, :, :VC]
    return out


# revision 16
# speedup vs baseline: 10.8525x; 4.5554x over previous
"""ChebConv (K=4) Trainium2 kernel: 8-core row-sharded SpMM + dense contraction.

v1 dataflow (all-bf16, SBUF-resident):
  Table layout: two regions, H0 = per-core rows [0,4096) (8*4096=32768 rows)
  and H1 = per-core rows [4096,6272) (8*2176=17408 rows). The int16 gather
  index splits exactly at the H0/H1 tensor boundary, and each AllGather step
  is two collectives (H0 issued after tile 31 so it overlaps the tail of the
  producing step; H1 after tile 48).
  Per 128-row tile: dma_gather edge columns (bf16, 512B rows) from H0/H1
  tables with per-core actual counts in a Q7 register (trailing pads are
  idx=-1 and never fetched); scatter-reduce into the tile's rows with
  per-128-slot indicator matmuls (mv = (iota==rl)*v, bf16) accumulating in
  PSUM. The Chebyshev subtract is folded into the same PSUM group via a
  leading (-identity) @ x_{k-2} matmul; x_{k-2} is read from the SBUF-
  resident bf16 x_k buffers (4 x 3.2MB), which also feed the contraction.
  Contraction (interleaved with step 3 per 4-tile block): PE-transpose
  x_k tiles to feature-major bf16, out[b].T = sum_k W_k.T @ x_k[b].T with
  bf16 weights, bias via tensor_scalar, fp32 out.
"""

import sys

sys.path.insert(0, "/opt/trn_rl_repo")

import numpy as np
import ml_dtypes

V = 50000
E = 800000
B, CIN, COUT, K = 4, 64, 128, 4
NC = 8
VC = V // NC              # 6250
VCP = 6272                # 49*128 padded rows per core
TILES = VCP // 128        # 49
F = B * CIN               # 256
H0R = 4096                # per-core rows in table region H0 (32 tiles)
H1R = VCP - H0R           # 2176 per-core rows in H1 (17 tiles)
H1AR = 1536               # H1 sub-region a (tiles 32-43), AG'd after tile 43
H1BR = H1R - H1AR         # 640, sub-region b (tiles 44-48), AG'd after tile 48
NH0 = NC * H0R            # 32768
NH1 = NC * H1R            # 17408
T0 = H0R // 128           # 32 tiles in H0
T1A = (H0R + H1AR) // 128 # 44: first tile after H1a
BF16 = ml_dtypes.bfloat16
SEGCAP = 1536


# ---------------- host-side preprocessing ----------------

def _ceil128(n):
    return max(128, -(-n // 128) * 128)


def _segs_of(total, base):
    """Chop [base, base+total) into SEGCAP pieces -> [(off, len), ...]."""
    out = []
    done = 0
    while done < total:
        m = min(SEGCAP, total - done)
        out.append((base + done, m))
        done += m
    return out


def preprocess(rows, cols, vals):
    """Per (core, tile): lo/hi column-sorted halves (lo -> H0 table,
    hi -> H1 table), static sizes padded to per-tile max over cores,
    per-core actual counts per gather segment, gather indices (pads=-1),
    per-chunk indicator metadata (bf16)."""
    rows = np.asarray(rows).astype(np.int64)
    cols = np.asarray(cols).astype(np.int64)
    vals = np.asarray(vals, dtype=np.float32)

    cc = cols // VC
    rr = cols % VC
    is_lo = rr < H0R
    remap = np.where(is_lo, cc * H0R + rr, cc * H1R + (rr - H0R))

    per_core = []
    nlo = np.zeros((NC, TILES), np.int64)
    nhi = np.zeros((NC, TILES), np.int64)
    for c in range(NC):
        m = (rows >= c * VC) & (rows < (c + 1) * VC)
        r = rows[m] - c * VC
        cg = remap[m]
        lo = is_lo[m]
        vv = vals[m]
        t_of = r // 128
        tiles = []
        for t in range(TILES):
            sel = t_of == t
            rt, ct, vt, lt = r[sel] - t * 128, cg[sel], vv[sel], lo[sel]

            def srt(rr2, cc2, vv2):
                o = np.argsort(cc2, kind="stable")
                return rr2[o], cc2[o], vv2[o]

            tl = srt(rt[lt], ct[lt], vt[lt])
            th = srt(rt[~lt], ct[~lt], vt[~lt])
            nlo[c, t] = len(tl[0])
            nhi[c, t] = len(th[0])
            tiles.append((tl, th))
        per_core.append(tiles)

    NLOS = [_ceil128(int(nlo[:, t].max())) for t in range(TILES)]
    NHIS = [_ceil128(int(nhi[:, t].max())) for t in range(TILES)]
    CPTS = [(NLOS[t] + NHIS[t]) // 128 for t in range(TILES)]
    SPT16S = [(NLOS[t] + NHIS[t]) // 16 for t in range(TILES)]
    NCH = sum(CPTS)
    NIDX = sum(SPT16S)
    # static segment table: per tile, lo segs then hi segs
    SEGS = []          # list per tile: [(off, len, is_hi), ...]
    for t in range(TILES):
        s = [(o, n, False) for o, n in _segs_of(NLOS[t], 0)]
        s += [(o, n, True) for o, n in _segs_of(NHIS[t], NLOS[t])]
        SEGS.append(s)
    NSEG = sum(len(s) for s in SEGS)

    cores = []
    for c in range(NC):
        idx = np.full((128, NIDX), -1, dtype=np.int16)
        rl = np.zeros((128, NCH), dtype=np.float32)
        v1 = np.zeros((128, NCH), dtype=np.float32)
        cnt = np.zeros((1, NSEG), dtype=np.int32)
        io = 0
        ch0 = 0
        si = 0
        for t in range(TILES):
            (rlo, clo, vlo), (rhi, chi, vhi) = per_core[c][t]
            NLO, NHI = NLOS[t], NHIS[t]
            SPT16, CPT = SPT16S[t], CPTS[t]
            ns = NLO + NHI
            rw = np.zeros(ns, np.float32)
            cw = np.full(ns, -1, np.int64)
            vw = np.zeros(ns, np.float32)
            n = len(rlo)
            rw[:n], cw[:n], vw[:n] = rlo, clo, vlo
            nh = len(rhi)
            rw[NLO : NLO + nh], cw[NLO : NLO + nh], vw[NLO : NLO + nh] = rhi, chi, vhi
            # per-seg actual counts (>=1: dummy valid idx 0 if empty)
            for off, ln, is_hi in SEGS[t]:
                na = n if not is_hi else nh
                base = 0 if not is_hi else NLO
                k = int(np.clip(na - (off - base), 0, ln))
                if k == 0:
                    cw[off] = 0
                    k = 1
                cnt[0, si] = k
                si += 1
            # gather idx, 16-wrapped, replicated across the 8 Q7 groups
            w = cw.reshape(SPT16, 16).T.astype(np.int16)
            idx[:, io : io + SPT16] = np.tile(w, (8, 1))
            rl[:, ch0 : ch0 + CPT] = rw.reshape(CPT, 128).T
            v1[:, ch0 : ch0 + CPT] = vw.reshape(CPT, 128).T
            io += SPT16
            ch0 += CPT
        cores.append(dict(idx=idx, cnt=cnt, rl=rl, v1=v1, v2=2.0 * v1))
    return cores, NLOS, NHIS, SEGS


def host_inputs(x, lap_rows, lap_cols, lap_vals, weight, bias):
    x0 = np.ascontiguousarray(np.transpose(x, (2, 0, 1)).reshape(V, F)).astype(np.float32)
    x0g = x0.astype(BF16)
    tab0h0 = np.zeros((NH0, F), dtype=BF16)
    tab0h1 = np.zeros((NH1, F), dtype=BF16)
    slices = []
    for c in range(NC):
        sl = np.zeros((VCP, F), dtype=BF16)
        sl[:VC] = x0g[c * VC : (c + 1) * VC]
        tab0h0[c * H0R : (c + 1) * H0R] = sl[:H0R]
        tab0h1[c * H1R : (c + 1) * H1R] = sl[H0R:]
        slices.append(sl)
    cores, NLOS, NHIS, SEGS = preprocess(lap_rows, lap_cols, lap_vals)

    consts = np.zeros((128, 384), dtype=np.float32)
    consts[:, 0:128] = np.eye(128)
    consts[:, 128:256] = -np.eye(128)
    consts[:, 256:384] = np.broadcast_to(np.arange(128, dtype=np.float32)[None, :], (128, 128))
    consts = consts.astype(BF16)

    wlo = np.zeros((128, K * COUT), np.float32)
    whi = np.zeros((128, K * COUT), np.float32)
    for k in range(K):
        wlo[0:64, k * COUT : (k + 1) * COUT] = weight[k]
        whi[64:128, k * COUT : (k + 1) * COUT] = weight[k]
    bias_t = np.asarray(bias, np.float32).reshape(128, 1)
    in_maps = []
    for c in range(NC):
        in_maps.append(
            dict(
                x0slice=slices[c],
                tab0h0=tab0h0,
                tab0h1=tab0h1,
                idx=cores[c]["idx"],
                cnt=cores[c]["cnt"],
                rl=cores[c]["rl"],
                nrl=-cores[c]["rl"],
                v1=cores[c]["v1"],
                v2=cores[c]["v2"],
                consts=consts,
                wlo=wlo.astype(BF16),
                whi=whi.astype(BF16),
                bias=bias_t,
            )
        )
    return in_maps, NLOS, NHIS, SEGS


# ---------------- device module ----------------

_CACHE = {}
_SCOPES = False


def build_module(NLOS, NHIS, SEGS, sim=False, NQ=4):
    NLOS, NHIS = tuple(NLOS), tuple(NHIS)
    key = (NLOS, NHIS, sim, NQ, _SCOPES)
    if key in _CACHE:
        return _CACHE[key]
    from concourse import bass, mybir, bacc
    import concourse.tile as tile

    CPTS = [(NLOS[t] + NHIS[t]) // 128 for t in range(TILES)]
    SPT16S = [(NLOS[t] + NHIS[t]) // 16 for t in range(TILES)]
    CHOFF = np.concatenate([[0], np.cumsum(CPTS)]).astype(int)
    IDXOFF = np.concatenate([[0], np.cumsum(SPT16S)]).astype(int)
    NCH = int(CHOFF[-1])
    NIDX = int(IDXOFF[-1])
    NSEG = sum(len(s) for s in SEGS)
    SEGOFF = np.concatenate([[0], np.cumsum([len(s) for s in SEGS])]).astype(int)
    CPT_MAX = max(CPTS)
    f32, i16, i32 = mybir.dt.float32, mybir.dt.int16, mybir.dt.int32
    gdt = mybir.dt.bfloat16

    nc = bacc.Bacc("TRN2", target_bir_lowering=False, debug=False,
                   num_devices=1 if sim else NC, num_swdge_queues=NQ)

    x0slice = nc.dram_tensor("x0slice", [VCP, F], gdt, kind="ExternalInput")
    tab0h0 = nc.dram_tensor("tab0h0", [NH0, F], gdt, kind="ExternalInput")
    tab0h1 = nc.dram_tensor("tab0h1", [NH1, F], gdt, kind="ExternalInput")
    idx_in = nc.dram_tensor("idx", [128, NIDX], i16, kind="ExternalInput")
    cnt_in = nc.dram_tensor("cnt", [1, NSEG], i32, kind="ExternalInput")
    rl_in = nc.dram_tensor("rl", [128, NCH], f32, kind="ExternalInput")
    nrl_in = nc.dram_tensor("nrl", [128, NCH], f32, kind="ExternalInput")
    v1_in = nc.dram_tensor("v1", [128, NCH], f32, kind="ExternalInput")
    v2_in = nc.dram_tensor("v2", [128, NCH], f32, kind="ExternalInput")
    consts_in = nc.dram_tensor("consts", [128, 384], gdt, kind="ExternalInput")
    wlo_in = nc.dram_tensor("wlo", [128, K * COUT], gdt, kind="ExternalInput")
    whi_in = nc.dram_tensor("whi", [128, K * COUT], gdt, kind="ExternalInput")
    bias_in = nc.dram_tensor("bias", [128, 1], f32, kind="ExternalInput")
    out_t = nc.dram_tensor("out", [B, COUT, VCP], f32, kind="ExternalOutput")

    creg = [nc.alloc_register(mybir.EngineType.Pool, name=f"cnt{i}") for i in range(2)]

    with tile.TileContext(nc) as tc:
        with (
            tc.tile_pool(name="pers", bufs=1) as pers,
            tc.tile_pool(name="gpool", bufs=4) as gpool,
            tc.tile_pool(name="mval", bufs=20) as mvpool,
            tc.tile_pool(name="actt", bufs=6) as actp,
            tc.tile_pool(name="spmm_ps", bufs=3, space="PSUM") as pspool,
            tc.tile_pool(name="tp_ps", bufs=2, space="PSUM") as tppool,
            tc.tile_pool(name="out_ps", bufs=2, space="PSUM") as popool,
            tc.tile_pool(name="xt", bufs=10) as xtpool,
            tc.tile_pool(name="obuf", bufs=3) as obpool,
            tc.tile_pool(name="dram", bufs=1, space="DRAM") as dram,
        ):
            import contextlib
            scope = nc.named_scope if _SCOPES else (lambda name: contextlib.nullcontext())

            idx_t = pers.tile([128, NIDX], i16)
            nc.sync.dma_start(idx_t[:], idx_in[:])
            cnt_t = pers.tile([1, NSEG], i32)
            nc.sync.dma_start(cnt_t[:], cnt_in[:])
            rl_t = pers.tile([128, NCH], f32)
            nc.sync.dma_start(rl_t[:], rl_in[:])
            nrl_t = pers.tile([128, NCH], f32)
            nc.sync.dma_start(nrl_t[:], nrl_in[:])
            v1_t = pers.tile([128, NCH], f32)
            nc.sync.dma_start(v1_t[:], v1_in[:])
            v2_t = pers.tile([128, NCH], f32)
            nc.sync.dma_start(v2_t[:], v2_in[:])
            consts_t = pers.tile([128, 384], gdt)
            nc.sync.dma_start(consts_t[:], consts_in[:])
            wlo_t = pers.tile([128, K * COUT], gdt)
            nc.sync.dma_start(wlo_t[:], wlo_in[:])
            whi_t = pers.tile([128, K * COUT], gdt)
            nc.sync.dma_start(whi_t[:], whi_in[:])
            bias_t = pers.tile([128, 1], f32)
            nc.sync.dma_start(bias_t[:], bias_in[:])
            ident = consts_t[:, 0:128]
            negident = consts_t[:, 128:256]
            iota = consts_t[:, 256:384]

            # SBUF-resident x_k (bf16): [p, t, f]
            xk_sb = [pers.tile([128, TILES * F], gdt, name=f"xk{k}") for k in range(K)]
            nc.sync.dma_start(
                xk_sb[0][:].rearrange("p (t f) -> p t f", f=F),
                x0slice[:].rearrange("(t p) f -> p t f", p=128),
            )
            # zero-fill gather pool bufs once (pad slots are never written by
            # the count-limited gathers; stale-garbage * mv=0 must stay finite)
            for _ in range(4):
                zt = gpool.tile([128, CPT_MAX * F], gdt, tag="G")
                nc.vector.memset(zt[:], 0.0)

            bncg = [dram.tile([VCP, F], gdt, name=f"bncg{i}", tag=f"bncg{i}") for i in range(2)]
            tabs = []
            for i in range(2):
                shared = "Local" if sim else "Shared"
                tabs.append((
                    dram.tile([NH0, F], gdt, name=f"tab{i+1}h0", tag=f"tab{i+1}h0", addr_space=shared),
                    dram.tile([NH1, F], gdt, name=f"tab{i+1}h1", tag=f"tab{i+1}h1", addr_space=shared),
                ))

            # ---------- contraction ----------
            def phase2_vblock(vb):
                with scope("p2"):
                    nt = min(4, TILES - vb * 4)
                    v0, nv = vb * 4 * 128, nt * 128
                    xts = []
                    for k in range(K):
                        xt_lo = xtpool.tile([128, 512], gdt, tag="xtlo")
                        xt_hi = xtpool.tile([128, 512], gdt, tag="xthi")
                        for q in range(nt):
                            for h in range(2):
                                tp = tppool.tile([128, 128], gdt, space="PSUM")
                                nc.tensor.transpose(
                                    out=tp[:],
                                    in_=xk_sb[k][:, (4 * vb + q) * F + h * 128 : (4 * vb + q) * F + (h + 1) * 128],
                                    identity=ident,
                                )
                                dst = xt_lo if h == 0 else xt_hi
                                nc.any.tensor_copy(out=dst[:, q * 128 : (q + 1) * 128], in_=tp[:])
                        xts.append((xt_lo, xt_hi))
                    for b in range(B):
                        h, off = divmod(b, 2)
                        off *= 64
                        wt = wlo_t if off == 0 else whi_t
                        po = popool.tile([128, 512], f32, space="PSUM")
                        for k in range(K):
                            xt = xts[k][h]
                            nc.tensor.matmul(
                                out=po[:, :nv], lhsT=wt[off : off + 64, k * COUT : (k + 1) * COUT],
                                rhs=xt[off : off + 64, :nv], start=(k == 0), stop=(k == K - 1),
                            )
                        ob = obpool.tile([128, 512], f32, tag="ob")
                        nc.any.tensor_scalar_add(ob[:, :nv], po[:, :nv], bias_t[:, 0:1])
                        nc.sync.dma_start(out_t[b, :, v0 : v0 + nv], ob[:, :nv])

            # ---------- SpMM steps ----------
            for k in (1, 2, 3):
                src_lo, src_hi = (tab0h0, tab0h1) if k == 1 else tabs[k - 2]
                vmeta = v1_t if k == 1 else v2_t
                prev = None if k == 1 else xk_sb[k - 2]
                ctx = scope(f"step{k}")
                ctx.__enter__()
                for t in range(TILES):
                    NLO, NHI, CPT = NLOS[t], NHIS[t], CPTS[t]
                    gt = gpool.tile([128, CPT_MAX * F], gdt, tag="G")
                    c0 = int(IDXOFF[t])
                    for si, (off, n, hi) in enumerate(SEGS[t]):
                        sg = int(SEGOFF[t]) + si
                        reg = creg[sg % 2]
                        nc.gpsimd.reg_load(reg, cnt_t[0:1, sg : sg + 1])
                        nc.gpsimd.dma_gather(
                            out_ap=gt[:, off * 2 : (off + n) * 2].rearrange(
                                "p (j f) -> p j f", f=F),
                            in_ap=(src_hi if hi else src_lo)[:],
                            idxs_ap=idx_t[:, c0 + off // 16 : c0 + (off + n) // 16],
                            num_idxs=n, num_idxs_reg=reg, elem_size=F,
                            single_packet=False, queue_num=(t * 3 + si) % NQ,
                        )
                    ps = pspool.tile([128, F], f32, space="PSUM")
                    if k > 1:
                        nc.tensor.matmul(
                            out=ps[:], lhsT=negident, rhs=prev[:, t * F : (t + 1) * F],
                            start=True, stop=False,
                        )
                    for j in range(CPT):
                        ch = int(CHOFF[t]) + j
                        mv = mvpool.tile([128, 128], gdt)
                        if ch % 4 == 3:
                            # offload ~25% of indicator builds to the idle ACT
                            # engine: mv = v * relu(1 - (iota - rl)^2)
                            u = actp.tile([128, 128], gdt)
                            nc.scalar.activation(u[:], iota,
                                                 mybir.ActivationFunctionType.Square,
                                                 bias=nrl_t[:, ch : ch + 1])
                            r = actp.tile([128, 128], gdt)
                            nc.scalar.activation(r[:], u[:],
                                                 mybir.ActivationFunctionType.Relu,
                                                 bias=1.0, scale=-1.0)
                            nc.scalar.mul(mv[:], r[:], vmeta[:, ch : ch + 1])
                        else:
                            nc.vector.tensor_scalar(
                                out=mv[:], in0=iota,
                                scalar1=rl_t[:, ch : ch + 1], scalar2=vmeta[:, ch : ch + 1],
                                op0=mybir.AluOpType.is_equal, op1=mybir.AluOpType.mult,
                            )
                        nc.tensor.matmul(
                            out=ps[:], lhsT=mv[:], rhs=gt[:, j * F : (j + 1) * F],
                            start=(j == 0 and k == 1), stop=(j == CPT - 1),
                        )
                    nc.scalar.copy(out=xk_sb[k][:, t * F : (t + 1) * F], in_=ps[:])
                    if k < 3:
                        nc.sync.dma_start(
                            bncg[k - 1][t * 128 : (t + 1) * 128, :],
                            xk_sb[k][:, t * F : (t + 1) * F])
                        if t == T0 - 1 or t == TILES - 1:
                            h = 0 if t == T0 - 1 else 1
                            agin = bncg[k - 1][0:H0R, :] if h == 0 else bncg[k - 1][H0R:VCP, :]
                            agout = tabs[k - 1][h][:]
                            with scope(f"ag{k}{'ab'[h]}"):
                                if not sim:
                                    nc.gpsimd.collective_compute(
                                        "AllGather", mybir.AluOpType.bypass,
                                        replica_groups=[list(range(NC))],
                                        ins=[agin.opt()], outs=[agout.opt()],
                                    )
                    if k == 3 and t % 4 == 3:
                        phase2_vblock(t // 4)
                ctx.__exit__(None, None, None)

            phase2_vblock(12)  # tail tile 48

    nc.compile()
    _CACHE[key] = nc
    return nc


# ---------------- entry point ----------------

def kernel(x, lap_rows, lap_cols, lap_vals, weight, bias):
    from concourse.bass_utils import run_bass_kernel_spmd

    x = np.asarray(x, np.float32)
    weight = np.asarray(weight, np.float32)
    bias = np.asarray(bias, np.float32)
    in_maps, NLOS, NHIS, SEGS = host_inputs(x, lap_rows, lap_cols, lap_vals, weight, bias)
    nc = build_module(NLOS, NHIS, SEGS)
    res = run_bass_kernel_spmd(nc, in_maps, core_ids=list(range(NC)))
    out = np.empty((B, COUT, V), np.float32)
    for c in range(NC):
        out[:, :, c * VC : (c + 1) * VC] = res.results[c]["out"][:, :, :VC]
    return out
